# revision 84
# baseline (speedup 1.0000x reference)
"""Multi-head causal attention on 8 Trainium2 NeuronCores.

Sharding: core c -> batch b = c // 4, head group g = c % 4 (4 of 16 heads).
Each core computes q/k/v for its 4 heads, causal softmax attention, and a
partial output  z_norm @ W_O[heads]  of shape [S, D].  Host sums the 4
head-group partials per batch and adds b_O.

The projections (A) and attention (B) are fully interleaved.  The
attention softmax is ACT-engine-bound (exp runs only on ACT) while the
projections are PE-bound with ACT idle, so B(head, chunk) is emitted
between A(head', half-chunk) blocks: attention for chunk C of head h
runs as soon as h's q/k through chunk C exist.  PE stays the global
bottleneck (~94% busy in the timeline model) instead of the phases
being serially engine-bound.

Implementation notes:
 - all matmul operands fp16 (host-quantized inputs), PSUM accumulates
   f32; fp16 keeps the full PE rate and halves DMA/SBUF
 - ONE shared 8-bank PSUM pool (single tile tag) for projection
   accumulators, score tiles, PV accumulators and transposes; every tile
   is <= 1 bank so emission order gives natural ring rotation
 - v is produced directly in natural [s, e] layout by the projection
   (lhsT = x.T slice) so attention needs no v transposes; its bias comes
   in pre-broadcast from the host since it varies along the free dim
 - q lives in a 2-chunk rolling buffer (scores consume it immediately),
   freeing SBUF for a third expT buffer so PV trails its scores' exps by
   ~2 sections of ACT lead
 - q/k bias+scale+cast on the DVE (tensor_scalar), exp on ACT, causal
   masking via Pool-engine zeroing of the exp'd diagonal 128-block, and
   diagonal-chunk score matmuls restricted to the valid causal wedge
 - z row-normalization (DVE recip+scale) decoupled from the in-order PE
   stream by deferring each z transpose four PV groups
 - x streamed in half-chunks of 256 columns; W_QKV uploaded in a
   wave-major column layout and shipped q/k-columns-first so each hc=0
   d-major sub-wave consumes W/x slices in DMA-arrival order (v weights
   aren't needed until the vnat sub-wave)
 - per-DMA cost is ~1.3us queue + bandwidth, so few big DMAs: W in 2-d
   slices, x one DMA per half-chunk, output one DMA per 128-row tile
   (the last row split per-chunk to shorten the kernel tail)
"""

import sys

for _p in ("/opt/trn_rl_repo",):
    if _p not in sys.path:
        sys.path.insert(0, _p)

import numpy as np

import concourse.bass as bass
from concourse import bacc
import concourse.mybir as mybir
import concourse.tile as tile
from concourse.bass_utils import run_bass_kernel_spmd
from concourse.masks import make_identity

F32 = mybir.dt.float32
F16 = mybir.dt.float16

B, S, D, H, E = 2, 2048, 2048, 16, 128
HL = 4          # heads per core
NCORES = 8
P = 128         # partitions
CH = 512        # attention i-chunk
HC = 256        # projection half-chunk (x streaming granularity)
S_T = S // P    # 16 seq tiles
S_C = S // CH   # 4 attention chunks
N_HC = S // HC  # 8 projection half-chunks
D_T = D // P    # 16 model-dim subtiles
D_C = D // CH   # 4 model-dim chunks
INV_SQRT_E = 1.0 / float(np.sqrt(E))

AF = mybir.ActivationFunctionType


def _wcol(m, h):
    """Column of group (m, h) in the wave-major W_QKV layout."""
    return (h // 2) * (3 * 2 * E) + m * (2 * E) + (h % 2) * E


def _trace_kernel(tc, xt, wqkv, wo, bqkv, bvb, outp):
    nc = tc.nc
    ts = bass.ts

    xt3 = xt.rearrange("(o p) s -> p o s", p=P)            # [128, 16, 2048]
    w4 = wqkv.rearrange("(o p) q -> p o q", p=P)           # [128, 16, 1536]
    wo3 = wo.rearrange("(h p) d -> p h d", p=P)            # [128, 4, 2048]
    out3 = outp.rearrange("(t p) d -> t p d", p=P)         # [16, 128, 2048]

    from contextlib import ExitStack

    with ExitStack() as top:
        const_pool = top.enter_context(tc.tile_pool(name="consts", bufs=1))
        qkvpool = top.enter_context(tc.tile_pool(name="qkvres", bufs=1))
        zpool = top.enter_context(tc.tile_pool(name="zT", bufs=1))
        wopool = top.enter_context(tc.tile_pool(name="wo", bufs=1))

        # k/v persist whole-sequence; q is consumed by scores as soon as its
        # chunk completes, so a 2-chunk rolling buffer suffices (saves SBUF)
        kvT = qkvpool.tile([P, 2 * HL, S], F16)    # [e, {k: h, v: HL+h}, s]
        qroll = qkvpool.tile([P, HL, 2, CH], F16)
        zT = zpool.tile([P, HL, S], F16)
        wo_sb = wopool.tile([P, HL, D], F16)

        def qkv_target(m, h, hc):
            if m == 0:
                return qroll[:, h, (hc // 2) % 2, (hc % 2) * HC : (hc % 2 + 1) * HC]
            return kvT[:, (m - 1) * HL + h, hc * HC : (hc + 1) * HC]

        identity = const_pool.tile([P, P], F16)
        make_identity(nc, identity)

        biases = const_pool.tile([P, 2, HL], F32)
        # b_V broadcast across partitions on host (v is produced in natural
        # [s, e] layout, so its bias varies along the free dim)
        bvb_sb = const_pool.tile([P, HL, E], F32)

        with ExitStack() as pab:
            wpool = pab.enter_context(tc.tile_pool(name="wqkv", bufs=1))
            xpool = pab.enter_context(tc.tile_pool(name="xchunk", bufs=2))
            vnp = pab.enter_context(tc.tile_pool(name="vnat", bufs=4))
            epool = pab.enter_context(tc.tile_pool(name="expT", bufs=3))
            zsp = pab.enter_context(tc.tile_pool(name="zsb", bufs=6))
            small = pab.enter_context(tc.tile_pool(name="small", bufs=4))
            ps = pab.enter_context(tc.tile_pool(name="ps", bufs=8, space="PSUM"))

            w_sb = wpool.tile([P, D_T, 3 * 2 * E * 2], F16)  # [p, d, 1536]
            xcs = {}

            # ---- DMA: warmup stream ordered by first use — wave-A q/k
            # columns + x half-chunk 0 first (in 2-d-slice pieces so the
            # d-major sub-waves consume in arrival order), then wave-A v
            # columns, then the same for wave B.  v weights aren't needed
            # until the vnat sub-wave, so the PE starts ~1 MB of DMA sooner.
            WA = 3 * 2 * E   # 768 columns per head-pair wave
            QK = 2 * 2 * E   # first 512 of those are the q/k columns
            xcs[0] = xpool.tile([P, D_T, HC], F16, name="xc")
            # scalar (ACT) HWDGE queue for the small/off-path transfers
            nc.scalar.dma_start(biases, bqkv.rearrange("(m h p) -> p m h", m=2, p=P))
            nc.scalar.dma_start(bvb_sb, bvb.rearrange("(p h e) -> p h e", p=P, h=HL))
            nc.scalar.dma_start(wo_sb, wo3)
            for piece in range(D_T // 2):
                sl = slice(2 * piece, 2 * piece + 2)
                nc.sync.dma_start(w_sb[:, sl, :QK], w4[:, sl, :QK])
                nc.sync.dma_start(xcs[0][:, sl, :], xt3[:, sl, 0:HC])
            for piece in range(D_T // 2):
                sl = slice(2 * piece, 2 * piece + 2)
                nc.sync.dma_start(w_sb[:, sl, QK:WA], w4[:, sl, QK:WA])

            def dma_wave_b():
                for piece in range(D_T // 4):
                    sl = slice(4 * piece, 4 * piece + 4)
                    nc.sync.dma_start(w_sb[:, sl, WA : WA + QK], w4[:, sl, WA : WA + QK])
                for piece in range(D_T // 4):
                    sl = slice(4 * piece, 4 * piece + 4)
                    nc.sync.dma_start(w_sb[:, sl, WA + QK :], w4[:, sl, WA + QK :])

            def emit_proj_group(m, h, hc, xc):
                pg = ps.tile([P, HC], F32, name="ps")
                for d in range(D_T):
                    nc.tensor.matmul(
                        pg,
                        w_sb[:, d, _wcol(m, h) : _wcol(m, h) + E],
                        xc[:, d, :],
                        start=(d == 0),
                        stop=(d == D_T - 1),
                    )
                # bias+scale+fp16 cast on DVE (ACT is reserved for exp)
                nc.vector.tensor_scalar(
                    qkv_target(m, h, hc),
                    pg,
                    INV_SQRT_E if m == 0 else 1.0,
                    biases[:, m, h, None],
                    op0=mybir.AluOpType.mult,
                    op1=mybir.AluOpType.add,
                )

            # v in natural [s=j, e] layout, produced directly by the
            # projection (lhsT = x.T slice, rhs = W_V slice) — no transposes
            v_augs = {}
            for lh in range(HL):
                v_aug = vnp.tile([P, S_T, E + 1], F16, name="v_aug")
                nc.gpsimd.memset(v_aug[:, :, E : E + 1], 1.0)
                v_augs[lh] = v_aug
            pending_z = []

            def emit_vnat(h, hc, xc):
                for a2 in range(HC // P):
                    jt = (HC // P) * hc + a2
                    pg = ps.tile([P, E], F32, name="ps")
                    for d in range(D_T):
                        nc.tensor.matmul(
                            pg,
                            xc[:, d, ts(a2, P)],
                            w_sb[:, d, _wcol(2, h) : _wcol(2, h) + E],
                            start=(d == 0),
                            stop=(d == D_T - 1),
                        )
                    nc.vector.tensor_add(
                        v_augs[h][:, jt, :E], pg, bvb_sb[:, h, :]
                    )

            def emit_scores(lh, c, expT, jts):
                qT = qroll[:, lh, c % 2, :]
                kT = kvT[:, lh, :]
                for jt in jts:
                    b = jt - S_C * c
                    sps = ps.tile([P, CH], F32, name="ps")
                    if b >= 0:
                        # diagonal chunk: cols < b*128 are never read by PV
                        nc.tensor.matmul(
                            sps[:, b * P :],
                            kT[:, ts(jt, P)],
                            qT[:, b * P :],
                            start=True,
                            stop=True,
                        )
                        nc.scalar.activation(
                            expT[:, jt, b * P :], sps[:, b * P :], AF.Exp
                        )
                        # zero the sub-diagonal of the 128-wide diag block
                        nc.gpsimd.affine_select(
                            out=expT[:, jt, ts(b, P)],
                            in_=expT[:, jt, ts(b, P)],
                            compare_op=mybir.AluOpType.is_ge,
                            fill=0.0,
                            base=0,
                            pattern=[[1, P]],
                            channel_multiplier=-1,
                        )
                    else:
                        nc.tensor.matmul(
                            sps,
                            kT[:, ts(jt, P)],
                            qT,
                            start=True,
                            stop=True,
                        )
                        nc.scalar.activation(expT[:, jt, :], sps, AF.Exp)

            def pop_ztrans(keep=1):
                while len(pending_z) > keep:
                    lh_p, i_p, z_p = pending_z.pop(0)
                    tpz = ps.tile([P, P], F16, name="ps")
                    nc.tensor.transpose(tpz, z_p, identity)
                    nc.vector.tensor_copy(zT[:, lh_p, ts(i_p, P)], tpz)

            def emit_pv_block(lh, c, expT):
                v_aug = v_augs[lh]
                for a in range(S_C):
                    i = S_C * c + a
                    z_ps = ps.tile([P, E + 1], F32, name="ps")
                    for jt in range(i + 1):
                        nc.tensor.matmul(
                            z_ps,
                            expT[:, jt, ts(a, P)],
                            v_aug[:, jt, :],
                            start=(jt == 0),
                            stop=(jt == i),
                        )
                    rec = small.tile([P, 1], F32, name="rec")
                    nc.vector.reciprocal(rec, z_ps[:, E : E + 1])
                    z_sb = zsp.tile([P, E], F16, name="z_sb")
                    nc.vector.tensor_scalar_mul(z_sb, z_ps[:, :E], rec)
                    pending_z.append((lh, i, z_sb))
                    # transpose of an EARLIER i-tile: its DVE recip/scale
                    # chain is hidden behind two PV groups' matmuls
                    pop_ztrans(keep=4)

            # ---- interleaved emission ----
            # hc=0 warmup: per head-pair, a q/k d-major sub-wave followed by
            # a vnat d-major sub-wave, each consuming its own W column range
            # in DMA-arrival order (q/k columns ship before v columns)
            for hpair in range(2):
                if hpair == 1:
                    dma_wave_b()
                hs = (2 * hpair, 2 * hpair + 1)
                pq = {h: ps.tile([P, HC], F32, name="ps") for h in hs}
                pk = {h: ps.tile([P, HC], F32, name="ps") for h in hs}
                for d in range(D_T):
                    for h in hs:
                        for m, pg in ((0, pq[h]), (1, pk[h])):
                            nc.tensor.matmul(
                                pg,
                                w_sb[:, d, _wcol(m, h) : _wcol(m, h) + E],
                                xcs[0][:, d, :],
                                start=(d == 0),
                                stop=(d == D_T - 1),
                            )
                for h in hs:
                    for m, pg in ((0, pq[h]), (1, pk[h])):
                        nc.vector.tensor_scalar(
                            qkv_target(m, h, 0),
                            pg,
                            INV_SQRT_E if m == 0 else 1.0,
                            biases[:, m, h, None],
                            op0=mybir.AluOpType.mult,
                            op1=mybir.AluOpType.add,
                        )
                pv = {
                    (h, a2): ps.tile([P, E], F32, name="ps")
                    for h in hs
                    for a2 in range(HC // P)
                }
                for d in range(D_T):
                    for h in hs:
                        for a2 in range(HC // P):
                            nc.tensor.matmul(
                                pv[(h, a2)],
                                xcs[0][:, d, ts(a2, P)],
                                w_sb[:, d, _wcol(2, h) : _wcol(2, h) + E],
                                start=(d == 0),
                                stop=(d == D_T - 1),
                            )
                for h in hs:
                    for a2 in range(HC // P):
                        nc.vector.tensor_add(
                            v_augs[h][:, a2, :E], pv[(h, a2)], bvb_sb[:, h, :]
                        )

            pending_pv = []
            expTs = {}
            xcs[1] = xpool.tile([P, D_T, HC], F16, name="xc")
            nc.sync.dma_start(xcs[1], xt3[:, :, ts(1, HC)])
            for hc in range(1, N_HC):
                C = hc // 2
                xc = xcs[hc]
                for h in range(HL):
                    if h == 1 and hc + 1 < N_HC:
                        # prefetch next half-chunk of x one section ahead so
                        # the next section's projections never wait on it
                        xcs[hc + 1] = xpool.tile([P, D_T, HC], F16, name="xc")
                        nc.sync.dma_start(xcs[hc + 1], xt3[:, :, ts(hc + 1, HC)])
                    for m in range(2):
                        emit_proj_group(m, h, hc, xc)
                    if hc % 2 == 0:
                        emit_vnat(h, hc, xc)
                        if len(pending_pv) >= 2:
                            lh_p, c_p = pending_pv.pop(0)
                            emit_pv_block(lh_p, c_p, expTs.pop((lh_p, c_p)))
                    else:
                        # diagonal tiles first: their exp -> Pool-zero chain
                        # is the longest per-tile latency and gates the first
                        # PV group; then spread the off-diagonal tiles
                        # between the other emissions so the 8-slot psum
                        # ring never sees a burst deeper than the ACT exp
                        # backlog it implies
                        n_jt = S_C * C + 4
                        jts = list(range(n_jt))
                        expT = epool.tile([P, S_T, CH], F16, name="expT")
                        expTs[(h, C)] = expT
                        emit_scores(h, C, expT, jts[: n_jt // 3])
                        emit_vnat(h, hc, xc)
                        emit_scores(h, C, expT, jts[n_jt // 3 : 2 * n_jt // 3])
                        # keep one extra PV in flight so its exps get ~2
                        # sections of ACT lead before the PE consumes them
                        if len(pending_pv) >= 2:
                            lh_p, c_p = pending_pv.pop(0)
                            emit_pv_block(lh_p, c_p, expTs.pop((lh_p, c_p)))
                        emit_scores(h, C, expT, jts[2 * n_jt // 3 :])
                        pending_pv.append((h, C))
            while pending_pv:
                lh_p, c_p = pending_pv.pop(0)
                emit_pv_block(lh_p, c_p, expTs.pop((lh_p, c_p)))
            pop_ztrans(keep=0)

        # ---------------- Phase C: output projection ----------------
        with ExitStack() as pc:
            ostage = pc.enter_context(tc.tile_pool(name="ostage", bufs=3))
            psC = pc.enter_context(tc.tile_pool(name="psC", bufs=4, space="PSUM"))

            for t in range(S_T):
                last = t == S_T - 1
                ot = ostage.tile([P, D], F16, name="ot")
                for dc in range(D_C):
                    ops = psC.tile([P, CH], F32, name="ops")
                    for lh in range(HL):
                        nc.tensor.matmul(
                            ops,
                            zT[:, lh, ts(t, P)],
                            wo_sb[:, lh, ts(dc, CH)],
                            start=(lh == 0),
                            stop=(lh == HL - 1),
                        )
                    # alternate drain engines so neither serializes the PE
                    if dc % 2 == 0:
                        nc.vector.tensor_copy(ot[:, ts(dc, CH)], ops)
                    else:
                        nc.scalar.activation(ot[:, ts(dc, CH)], ops, AF.Copy)
                    if last:
                        # final row: per-chunk DMAs overlap the copies so the
                        # kernel doesn't end on one full-row transfer
                        nc.sync.dma_start(out3[t, :, ts(dc, CH)], ot[:, ts(dc, CH)])
                if not last:
                    nc.sync.dma_start(out3[t], ot)


_NC_CACHE = {}
LAST_RESULTS = None


def _get_nc():
    if "nc" not in _NC_CACHE:
        nc = bacc.Bacc("TRN2", target_bir_lowering=False, debug=False)
        xt = nc.dram_tensor("xt", [D, S], F16, kind="ExternalInput")
        wqkv = nc.dram_tensor("wqkv", [D, 3 * HL * E], F16, kind="ExternalInput")
        wo = nc.dram_tensor("wo", [HL * E, D], F16, kind="ExternalInput")
        bqkv = nc.dram_tensor("bqkv", [2 * HL * E], F32, kind="ExternalInput")
        bvb = nc.dram_tensor("bvb", [P * HL * E], F32, kind="ExternalInput")
        outp = nc.dram_tensor("outp", [S, D], F16, kind="ExternalOutput")
        with tile.TileContext(nc) as tc:
            _trace_kernel(tc, xt, wqkv, wo, bqkv, bvb, outp)
        nc.compile()
        _NC_CACHE["nc"] = nc
    return _NC_CACHE["nc"]


def kernel(normalized_resid_pre, W_Q, W_K, W_V, W_O, b_Q, b_K, b_V, b_O):
    x = np.asarray(normalized_resid_pre, np.float32)
    W_Q = np.asarray(W_Q, np.float32)
    W_K = np.asarray(W_K, np.float32)
    W_V = np.asarray(W_V, np.float32)
    W_O = np.asarray(W_O, np.float32)
    b_Q = np.asarray(b_Q, np.float32)
    b_K = np.asarray(b_K, np.float32)
    b_V = np.asarray(b_V, np.float32)
    b_O = np.asarray(b_O, np.float32)

    nc = _get_nc()
    Wm = [W_Q, W_K, W_V]
    xts = [np.ascontiguousarray(x[b].T.astype(np.float16)) for b in range(B)]
    gmaps = []
    for g in range(NCORES // B):
        h0 = g * HL
        # wave-major column layout: [h-pair][m][h-within-pair][E]
        cols = []
        for hp in range(2):
            for m in range(3):
                for hh in range(2):
                    cols.append(Wm[m][h0 + 2 * hp + hh])
        gmaps.append(
            {
                "wqkv": np.ascontiguousarray(
                    np.concatenate(cols, 1).astype(np.float16)
                ),
                "wo": np.ascontiguousarray(
                    W_O[h0 : h0 + HL].reshape(HL * E, D).astype(np.float16)
                ),
                "bqkv": np.ascontiguousarray(
                    np.concatenate(
                        [
                            b_Q[h0 : h0 + HL].reshape(-1) * np.float32(INV_SQRT_E),
                            b_K[h0 : h0 + HL].reshape(-1),
                        ]
                    )
                ),
                "bvb": np.ascontiguousarray(
                    np.broadcast_to(b_V[h0 : h0 + HL][None], (P, HL, E)).reshape(
                        -1
                    )
                ),
            }
        )
    in_maps = []
    for core in range(NCORES):
        b, g = core // (NCORES // B), core % (NCORES // B)
        in_maps.append({"xt": xts[b], **gmaps[g]})

    res = run_bass_kernel_spmd(nc, in_maps, core_ids=list(range(NCORES)))
    global LAST_RESULTS
    LAST_RESULTS = res
    out = np.zeros((B, S, D), np.float32)
    for core in range(NCORES):
        out[core // (NCORES // B)] += res.results[core]["outp"]
    out += b_O[None, None, :]
    return out


# revision 88
# speedup vs baseline: 1.0032x; 1.0032x over previous
"""Multi-head causal attention on 8 Trainium2 NeuronCores.

Sharding: core c -> batch b = c // 4, head group g = c % 4 (4 of 16 heads).
Each core computes q/k/v for its 4 heads, causal softmax attention, and a
partial output  z_norm @ W_O[heads]  of shape [S, D].  Host sums the 4
head-group partials per batch and adds b_O.

The projections (A) and attention (B) are fully interleaved.  The
attention softmax is ACT-engine-bound (exp runs only on ACT) while the
projections are PE-bound with ACT idle, so B(head, chunk) is emitted
between A(head', half-chunk) blocks: attention for chunk C of head h
runs as soon as h's q/k through chunk C exist.  PE stays the global
bottleneck (~94% busy in the timeline model) instead of the phases
being serially engine-bound.

Implementation notes:
 - all matmul operands fp16 (host-quantized inputs), PSUM accumulates
   f32; fp16 keeps the full PE rate and halves DMA/SBUF
 - ONE shared 8-bank PSUM pool (single tile tag) for projection
   accumulators, score tiles, PV accumulators and transposes; every tile
   is <= 1 bank so emission order gives natural ring rotation
 - v is produced directly in natural [s, e] layout by the projection
   (lhsT = x.T slice) so attention needs no v transposes; its bias comes
   in pre-broadcast from the host since it varies along the free dim
 - q lives in a 2-chunk rolling buffer (scores consume it immediately),
   freeing SBUF for a third expT buffer so PV trails its scores' exps by
   ~2 sections of ACT lead
 - q/k bias+scale+cast on the DVE (tensor_scalar), exp on ACT, causal
   masking via Pool-engine zeroing of the exp'd diagonal 128-block, and
   diagonal-chunk score matmuls restricted to the valid causal wedge
 - z row-normalization (DVE recip+scale) decoupled from the in-order PE
   stream by deferring each z transpose four PV groups
 - x streamed in half-chunks of 256 columns; W_QKV uploaded in a
   wave-major column layout and shipped q/k-columns-first so each hc=0
   d-major sub-wave consumes W/x slices in DMA-arrival order (v weights
   aren't needed until the vnat sub-wave)
 - per-DMA cost is ~1.3us queue + bandwidth, so few big DMAs: W in 2-d
   slices, x one DMA per half-chunk, output one DMA per 128-row tile
   (the last row split per-chunk to shorten the kernel tail)
"""

import sys

for _p in ("/opt/trn_rl_repo",):
    if _p not in sys.path:
        sys.path.insert(0, _p)

import numpy as np

import concourse.bass as bass
from concourse import bacc
import concourse.mybir as mybir
import concourse.tile as tile
from concourse.bass_utils import run_bass_kernel_spmd
from concourse.masks import make_identity

F32 = mybir.dt.float32
F16 = mybir.dt.float16

B, S, D, H, E = 2, 2048, 2048, 16, 128
HL = 4          # heads per core
NCORES = 8
P = 128         # partitions
CH = 512        # attention i-chunk
HC = 256        # projection half-chunk (x streaming granularity)
S_T = S // P    # 16 seq tiles
S_C = S // CH   # 4 attention chunks
N_HC = S // HC  # 8 projection half-chunks
D_T = D // P    # 16 model-dim subtiles
D_C = D // CH   # 4 model-dim chunks
INV_SQRT_E = 1.0 / float(np.sqrt(E))

AF = mybir.ActivationFunctionType


def _wcol(m, h):
    """Column of group (m, h) in the wave-major W_QKV layout."""
    return (h // 2) * (3 * 2 * E) + m * (2 * E) + (h % 2) * E


def _trace_kernel(tc, xt, wqkv, wo, bqkv, bvb, outp):
    nc = tc.nc
    ts = bass.ts

    xt3 = xt.rearrange("(o p) s -> p o s", p=P)            # [128, 16, 2048]
    w4 = wqkv.rearrange("(o p) q -> p o q", p=P)           # [128, 16, 1536]
    wo3 = wo.rearrange("(h p) d -> p h d", p=P)            # [128, 4, 2048]
    out3 = outp.rearrange("(t p) d -> t p d", p=P)         # [16, 128, 2048]

    from contextlib import ExitStack

    with ExitStack() as top:
        const_pool = top.enter_context(tc.tile_pool(name="consts", bufs=1))
        qkvpool = top.enter_context(tc.tile_pool(name="qkvres", bufs=1))
        zpool = top.enter_context(tc.tile_pool(name="zT", bufs=1))
        wopool = top.enter_context(tc.tile_pool(name="wo", bufs=1))

        # k/v persist whole-sequence; q is consumed by scores as soon as its
        # chunk completes, so a 2-chunk rolling buffer suffices (saves SBUF)
        kvT = qkvpool.tile([P, 2 * HL, S], F16)    # [e, {k: h, v: HL+h}, s]
        qroll = qkvpool.tile([P, HL, 2, CH], F16)
        zT = zpool.tile([P, HL, S], F16)
        wo_sb = wopool.tile([P, HL, D], F16)

        def qkv_target(m, h, hc):
            if m == 0:
                return qroll[:, h, (hc // 2) % 2, (hc % 2) * HC : (hc % 2 + 1) * HC]
            return kvT[:, (m - 1) * HL + h, hc * HC : (hc + 1) * HC]

        identity = const_pool.tile([P, P], F16)
        make_identity(nc, identity)

        biases = const_pool.tile([P, 2, HL], F32)
        # b_V broadcast across partitions on host (v is produced in natural
        # [s, e] layout, so its bias varies along the free dim)
        bvb_sb = const_pool.tile([P, HL, E], F32)

        with ExitStack() as pab:
            wpool = pab.enter_context(tc.tile_pool(name="wqkv", bufs=1))
            xpool = pab.enter_context(tc.tile_pool(name="xchunk", bufs=2))
            vnp = pab.enter_context(tc.tile_pool(name="vnat", bufs=4))
            epool = pab.enter_context(tc.tile_pool(name="expT", bufs=3))
            zsp = pab.enter_context(tc.tile_pool(name="zsb", bufs=6))
            small = pab.enter_context(tc.tile_pool(name="small", bufs=4))
            ps = pab.enter_context(tc.tile_pool(name="ps", bufs=8, space="PSUM"))

            w_sb = wpool.tile([P, D_T, 3 * 2 * E * 2], F16)  # [p, d, 1536]
            xcs = {}

            # ---- DMA: warmup stream ordered by first use — wave-A q/k
            # columns + x half-chunk 0 first (in 2-d-slice pieces so the
            # d-major sub-waves consume in arrival order), then wave-A v
            # columns, then the same for wave B.  v weights aren't needed
            # until the vnat sub-wave, so the PE starts ~1 MB of DMA sooner.
            WA = 3 * 2 * E   # 768 columns per head-pair wave
            QK = 2 * 2 * E   # first 512 of those are the q/k columns
            xcs[0] = xpool.tile([P, D_T, HC], F16, name="xc")
            # scalar (ACT) HWDGE queue for the small/off-path transfers
            nc.scalar.dma_start(biases, bqkv.rearrange("(m h p) -> p m h", m=2, p=P))
            nc.scalar.dma_start(bvb_sb, bvb.rearrange("(p h e) -> p h e", p=P, h=HL))
            nc.scalar.dma_start(wo_sb, wo3)
            for piece in range(D_T // 2):
                sl = slice(2 * piece, 2 * piece + 2)
                nc.sync.dma_start(w_sb[:, sl, :QK], w4[:, sl, :QK])
                if piece % 2 == 0:
                    sl4 = slice(2 * piece, 2 * piece + 4)
                    nc.sync.dma_start(xcs[0][:, sl4, :], xt3[:, sl4, 0:HC])
            for piece in range(D_T // 4):
                sl = slice(4 * piece, 4 * piece + 4)
                nc.sync.dma_start(w_sb[:, sl, QK:WA], w4[:, sl, QK:WA])

            def dma_wave_b():
                for piece in range(D_T // 8):
                    sl = slice(8 * piece, 8 * piece + 8)
                    nc.sync.dma_start(w_sb[:, sl, WA : WA + QK], w4[:, sl, WA : WA + QK])
                for piece in range(D_T // 8):
                    sl = slice(8 * piece, 8 * piece + 8)
                    nc.sync.dma_start(w_sb[:, sl, WA + QK :], w4[:, sl, WA + QK :])

            def emit_proj_group(m, h, hc, xc):
                pg = ps.tile([P, HC], F32, name="ps")
                for d in range(D_T):
                    nc.tensor.matmul(
                        pg,
                        w_sb[:, d, _wcol(m, h) : _wcol(m, h) + E],
                        xc[:, d, :],
                        start=(d == 0),
                        stop=(d == D_T - 1),
                    )
                # bias+scale+fp16 cast on DVE (ACT is reserved for exp)
                nc.vector.tensor_scalar(
                    qkv_target(m, h, hc),
                    pg,
                    INV_SQRT_E if m == 0 else 1.0,
                    biases[:, m, h, None],
                    op0=mybir.AluOpType.mult,
                    op1=mybir.AluOpType.add,
                )

            # v in natural [s=j, e] layout, produced directly by the
            # projection (lhsT = x.T slice, rhs = W_V slice) — no transposes
            v_augs = {}
            for lh in range(HL):
                v_aug = vnp.tile([P, S_T, E + 1], F16, name="v_aug")
                nc.gpsimd.memset(v_aug[:, :, E : E + 1], 1.0)
                v_augs[lh] = v_aug
            pending_z = []

            def emit_vnat(h, hc, xc):
                for a2 in range(HC // P):
                    jt = (HC // P) * hc + a2
                    pg = ps.tile([P, E], F32, name="ps")
                    for d in range(D_T):
                        nc.tensor.matmul(
                            pg,
                            xc[:, d, ts(a2, P)],
                            w_sb[:, d, _wcol(2, h) : _wcol(2, h) + E],
                            start=(d == 0),
                            stop=(d == D_T - 1),
                        )
                    nc.vector.tensor_add(
                        v_augs[h][:, jt, :E], pg, bvb_sb[:, h, :]
                    )

            def emit_scores(lh, c, expT, jts):
                qT = qroll[:, lh, c % 2, :]
                kT = kvT[:, lh, :]
                for jt in jts:
                    b = jt - S_C * c
                    sps = ps.tile([P, CH], F32, name="ps")
                    if b >= 0:
                        # diagonal chunk: cols < b*128 are never read by PV
                        nc.tensor.matmul(
                            sps[:, b * P :],
                            kT[:, ts(jt, P)],
                            qT[:, b * P :],
                            start=True,
                            stop=True,
                        )
                        nc.scalar.activation(
                            expT[:, jt, b * P :], sps[:, b * P :], AF.Exp
                        )
                        # zero the sub-diagonal of the 128-wide diag block
                        nc.gpsimd.affine_select(
                            out=expT[:, jt, ts(b, P)],
                            in_=expT[:, jt, ts(b, P)],
                            compare_op=mybir.AluOpType.is_ge,
                            fill=0.0,
                            base=0,
                            pattern=[[1, P]],
                            channel_multiplier=-1,
                        )
                    else:
                        nc.tensor.matmul(
                            sps,
                            kT[:, ts(jt, P)],
                            qT,
                            start=True,
                            stop=True,
                        )
                        nc.scalar.activation(expT[:, jt, :], sps, AF.Exp)

            def pop_ztrans(keep=1):
                while len(pending_z) > keep:
                    lh_p, i_p, z_p = pending_z.pop(0)
                    tpz = ps.tile([P, P], F16, name="ps")
                    nc.tensor.transpose(tpz, z_p, identity)
                    nc.vector.tensor_copy(zT[:, lh_p, ts(i_p, P)], tpz)

            def emit_pv_block(lh, c, expT):
                v_aug = v_augs[lh]
                for a in range(S_C):
                    i = S_C * c + a
                    z_ps = ps.tile([P, E + 1], F32, name="ps")
                    for jt in range(i + 1):
                        nc.tensor.matmul(
                            z_ps,
                            expT[:, jt, ts(a, P)],
                            v_aug[:, jt, :],
                            start=(jt == 0),
                            stop=(jt == i),
                        )
                    rec = small.tile([P, 1], F32, name="rec")
                    nc.vector.reciprocal(rec, z_ps[:, E : E + 1])
                    z_sb = zsp.tile([P, E], F16, name="z_sb")
                    nc.vector.tensor_scalar_mul(z_sb, z_ps[:, :E], rec)
                    pending_z.append((lh, i, z_sb))
                    # transpose of an EARLIER i-tile: its DVE recip/scale
                    # chain is hidden behind two PV groups' matmuls
                    pop_ztrans(keep=4)

            # ---- interleaved emission ----
            # hc=0 warmup: per head-pair, a q/k d-major sub-wave followed by
            # a vnat d-major sub-wave, each consuming its own W column range
            # in DMA-arrival order (q/k columns ship before v columns)
            for hpair in range(2):
                if hpair == 1:
                    dma_wave_b()
                hs = (2 * hpair, 2 * hpair + 1)
                pq = {h: ps.tile([P, HC], F32, name="ps") for h in hs}
                pk = {h: ps.tile([P, HC], F32, name="ps") for h in hs}
                for d in range(D_T):
                    for h in hs:
                        for m, pg in ((0, pq[h]), (1, pk[h])):
                            nc.tensor.matmul(
                                pg,
                                w_sb[:, d, _wcol(m, h) : _wcol(m, h) + E],
                                xcs[0][:, d, :],
                                start=(d == 0),
                                stop=(d == D_T - 1),
                            )
                for h in hs:
                    for m, pg in ((0, pq[h]), (1, pk[h])):
                        nc.vector.tensor_scalar(
                            qkv_target(m, h, 0),
                            pg,
                            INV_SQRT_E if m == 0 else 1.0,
                            biases[:, m, h, None],
                            op0=mybir.AluOpType.mult,
                            op1=mybir.AluOpType.add,
                        )
                pv = {
                    (h, a2): ps.tile([P, E], F32, name="ps")
                    for h in hs
                    for a2 in range(HC // P)
                }
                for d in range(D_T):
                    for h in hs:
                        for a2 in range(HC // P):
                            nc.tensor.matmul(
                                pv[(h, a2)],
                                xcs[0][:, d, ts(a2, P)],
                                w_sb[:, d, _wcol(2, h) : _wcol(2, h) + E],
                                start=(d == 0),
                                stop=(d == D_T - 1),
                            )
                for h in hs:
                    for a2 in range(HC // P):
                        nc.vector.tensor_add(
                            v_augs[h][:, a2, :E], pv[(h, a2)], bvb_sb[:, h, :]
                        )

            pending_pv = []
            expTs = {}
            xcs[1] = xpool.tile([P, D_T, HC], F16, name="xc")
            nc.sync.dma_start(xcs[1], xt3[:, :, ts(1, HC)])
            for hc in range(1, N_HC):
                C = hc // 2
                xc = xcs[hc]
                for h in range(HL):
                    if h == 1 and hc + 1 < N_HC:
                        # prefetch next half-chunk of x one section ahead so
                        # the next section's projections never wait on it
                        xcs[hc + 1] = xpool.tile([P, D_T, HC], F16, name="xc")
                        nc.sync.dma_start(xcs[hc + 1], xt3[:, :, ts(hc + 1, HC)])
                    for m in range(2):
                        emit_proj_group(m, h, hc, xc)
                    if hc % 2 == 0:
                        emit_vnat(h, hc, xc)
                        if len(pending_pv) >= 2:
                            lh_p, c_p = pending_pv.pop(0)
                            emit_pv_block(lh_p, c_p, expTs.pop((lh_p, c_p)))
                    else:
                        # diagonal tiles first: their exp -> Pool-zero chain
                        # is the longest per-tile latency and gates the first
                        # PV group; then spread the off-diagonal tiles
                        # between the other emissions so the 8-slot psum
                        # ring never sees a burst deeper than the ACT exp
                        # backlog it implies
                        n_jt = S_C * C + 4
                        jts = list(range(n_jt))
                        expT = epool.tile([P, S_T, CH], F16, name="expT")
                        expTs[(h, C)] = expT
                        emit_scores(h, C, expT, jts[: n_jt // 3])
                        emit_vnat(h, hc, xc)
                        emit_scores(h, C, expT, jts[n_jt // 3 : 2 * n_jt // 3])
                        # keep one extra PV in flight so its exps get ~2
                        # sections of ACT lead before the PE consumes them
                        if len(pending_pv) >= 2:
                            lh_p, c_p = pending_pv.pop(0)
                            emit_pv_block(lh_p, c_p, expTs.pop((lh_p, c_p)))
                        emit_scores(h, C, expT, jts[2 * n_jt // 3 :])
                        pending_pv.append((h, C))
            while pending_pv:
                lh_p, c_p = pending_pv.pop(0)
                emit_pv_block(lh_p, c_p, expTs.pop((lh_p, c_p)))
            pop_ztrans(keep=0)

        # ---------------- Phase C: output projection ----------------
        with ExitStack() as pc:
            ostage = pc.enter_context(tc.tile_pool(name="ostage", bufs=3))
            psC = pc.enter_context(tc.tile_pool(name="psC", bufs=4, space="PSUM"))

            for t in range(S_T):
                last = t == S_T - 1
                ot = ostage.tile([P, D], F16, name="ot")
                for dc in range(D_C):
                    ops = psC.tile([P, CH], F32, name="ops")
                    for lh in range(HL):
                        nc.tensor.matmul(
                            ops,
                            zT[:, lh, ts(t, P)],
                            wo_sb[:, lh, ts(dc, CH)],
                            start=(lh == 0),
                            stop=(lh == HL - 1),
                        )
                    # alternate drain engines so neither serializes the PE
                    if dc % 2 == 0:
                        nc.vector.tensor_copy(ot[:, ts(dc, CH)], ops)
                    else:
                        nc.scalar.activation(ot[:, ts(dc, CH)], ops, AF.Copy)
                    if last:
                        # final row: per-chunk DMAs overlap the copies so the
                        # kernel doesn't end on one full-row transfer
                        nc.sync.dma_start(out3[t, :, ts(dc, CH)], ot[:, ts(dc, CH)])
                if not last:
                    nc.sync.dma_start(out3[t], ot)


_NC_CACHE = {}
LAST_RESULTS = None


def _get_nc():
    if "nc" not in _NC_CACHE:
        nc = bacc.Bacc("TRN2", target_bir_lowering=False, debug=False)
        xt = nc.dram_tensor("xt", [D, S], F16, kind="ExternalInput")
        wqkv = nc.dram_tensor("wqkv", [D, 3 * HL * E], F16, kind="ExternalInput")
        wo = nc.dram_tensor("wo", [HL * E, D], F16, kind="ExternalInput")
        bqkv = nc.dram_tensor("bqkv", [2 * HL * E], F32, kind="ExternalInput")
        bvb = nc.dram_tensor("bvb", [P * HL * E], F32, kind="ExternalInput")
        outp = nc.dram_tensor("outp", [S, D], F16, kind="ExternalOutput")
        with tile.TileContext(nc) as tc:
            _trace_kernel(tc, xt, wqkv, wo, bqkv, bvb, outp)
        nc.compile()
        _NC_CACHE["nc"] = nc
    return _NC_CACHE["nc"]


def kernel(normalized_resid_pre, W_Q, W_K, W_V, W_O, b_Q, b_K, b_V, b_O):
    x = np.asarray(normalized_resid_pre, np.float32)
    W_Q = np.asarray(W_Q, np.float32)
    W_K = np.asarray(W_K, np.float32)
    W_V = np.asarray(W_V, np.float32)
    W_O = np.asarray(W_O, np.float32)
    b_Q = np.asarray(b_Q, np.float32)
    b_K = np.asarray(b_K, np.float32)
    b_V = np.asarray(b_V, np.float32)
    b_O = np.asarray(b_O, np.float32)

    nc = _get_nc()
    Wm = [W_Q, W_K, W_V]
    xts = [np.ascontiguousarray(x[b].T.astype(np.float16)) for b in range(B)]
    gmaps = []
    for g in range(NCORES // B):
        h0 = g * HL
        # wave-major column layout: [h-pair][m][h-within-pair][E]
        cols = []
        for hp in range(2):
            for m in range(3):
                for hh in range(2):
                    cols.append(Wm[m][h0 + 2 * hp + hh])
        gmaps.append(
            {
                "wqkv": np.ascontiguousarray(
                    np.concatenate(cols, 1).astype(np.float16)
                ),
                "wo": np.ascontiguousarray(
                    W_O[h0 : h0 + HL].reshape(HL * E, D).astype(np.float16)
                ),
                "bqkv": np.ascontiguousarray(
                    np.concatenate(
                        [
                            b_Q[h0 : h0 + HL].reshape(-1) * np.float32(INV_SQRT_E),
                            b_K[h0 : h0 + HL].reshape(-1),
                        ]
                    )
                ),
                "bvb": np.ascontiguousarray(
                    np.broadcast_to(b_V[h0 : h0 + HL][None], (P, HL, E)).reshape(
                        -1
                    )
                ),
            }
        )
    in_maps = []
    for core in range(NCORES):
        b, g = core // (NCORES // B), core % (NCORES // B)
        in_maps.append({"xt": xts[b], **gmaps[g]})

    res = run_bass_kernel_spmd(nc, in_maps, core_ids=list(range(NCORES)))
    global LAST_RESULTS
    LAST_RESULTS = res
    out = np.zeros((B, S, D), np.float32)
    for core in range(NCORES):
        out[core // (NCORES // B)] += res.results[core]["outp"]
    out += b_O[None, None, :]
    return out
